# revision 1
# baseline (speedup 1.0000x reference)
"""DGL-GCN (3-layer GraphConv, norm='both') on 8 Trainium2 NeuronCores.

Strategy (matches the dst-partition sharding hint):
  - Nodes/dst-rows are split across 8 cores (2500 each); edge lists are
    partitioned by destination core and sorted by destination block (128
    dst rows per block).
  - Per layer, each core dma_gathers the source-node feature rows for its
    edges (bf16), and scatter-adds them with one-hot matmuls on the tensor
    engine (PSUM-accumulated, exact fp32 accumulation).
  - The small 508x508 weights are replicated; per-core dense matmuls follow
    each aggregation.
  - Layer 1 exploits x = [broadcast(ce) | we]: the ce half is rank-1
    (outer(u, r) with r the normalized-adjacency row sums), so only the
    254-wide `we` half is gathered.
  - Layer 3 is aggregated after projecting to scalars (z = h2s @ W3), so
    only an 80KB AllGather (on-device collective) crosses cores mid-launch.
  - Two launches; the one large inter-layer exchange (h1s, 20MB) is done by
    the host between launches (gather/unshard glue).
"""

import math
import numpy as np
import ml_dtypes

import concourse.bass as bass
import concourse.bacc as bacc
import concourse.tile as tile
import concourse.mybir as mybir
from concourse.bass_utils import run_bass_kernel_spmd

dt = mybir.dt
AF = mybir.ActivationFunctionType
ALU = mybir.AluOpType

P = 8           # cores
G = 8           # gather chunk, tiles (128 edges each) for layers 1/2
G3 = 8          # gather chunk tiles for layer 3
import os as _os
GBUFS = int(_os.environ.get("K_GBUFS", "6"))
OBUFS = int(_os.environ.get("K_OBUFS", "4"))
MERGEA = bool(int(_os.environ.get("K_MERGEA", "1")))
PSA = int(_os.environ.get("K_PSA", "4"))
PAGG2 = int(_os.environ.get("K_PAGG2", "3"))
PZ2 = int(_os.environ.get("K_PZ2", "1"))
BLKB = int(_os.environ.get("K_BLKB", "3"))
PT1 = int(_os.environ.get("K_PT1", "2"))
PHT1 = int(_os.environ.get("K_PHT1", "2"))
PH1 = int(_os.environ.get("K_PH1", "2"))
PT2 = int(_os.environ.get("K_PT2", "2"))
PH2 = int(_os.environ.get("K_PH2", "2"))
NOBAR1 = bool(int(_os.environ.get("K_NOBAR1", "0")))
NOBAR2 = bool(int(_os.environ.get("K_NOBAR2", "0")))
BF16 = ml_dtypes.bfloat16


def rup(x, m):
    return (x + m - 1) // m * m


class Meta:
    """Static shape/tiling info shared by host prep and program builders."""

    def __init__(self, N, E, FD, IN, GLOVE, T_blk):
        self.N, self.E, self.FD, self.IN, self.GLOVE = N, E, FD, IN, GLOVE
        self.DCORE = N // P
        self.NB = math.ceil(self.DCORE / 128)
        self.DPAD = self.NB * 128
        self.F1 = rup(FD, 128)          # 256
        self.F2 = rup(IN, 128)          # 512
        self.NC1 = self.F1 // 128       # 2
        self.NC2 = self.F2 // 128       # 4
        self.GLP = rup(GLOVE, 128)      # 384
        self.NGC = self.GLP // 128      # 3
        self.NPAD = rup(N + 1, 512)     # 20480 (we_s / W_imgT row pad; +1 for b_img row)
        self.NT_N = self.NPAD // 128    # node tiles (160)
        self.NT_G = self.NPAD // 512    # 512-node groups (40)
        self.T_blk = T_blk              # tiles per dst block (uniform)
        self.ntiles = self.NB * T_blk   # real tiles per core
        self.nch = math.ceil(self.ntiles / G)
        self.NTILE = self.nch * G       # padded tiles per core
        self.nch3 = math.ceil(self.ntiles / G3)
        self.NTILE3 = self.nch3 * G3
        self.ZA = 128 // P              # z_all rearrange factor (16)
        assert self.DPAD % self.ZA == 0
        self.ZT = P * self.DPAD // 128  # z sbuf cols (160)


# ---------------------------------------------------------------- host prep

def _pack_idx(v):
    """Logical position i -> partition i%16, col i//16, replicated 8x."""
    assert len(v) % 16 == 0
    m = v.reshape(-1, 16).T.astype(np.int16)
    return np.ascontiguousarray(np.tile(m, (8, 1)))


def _pack_tile_major(v, fill, width=128):
    """Logical position i -> partition i%width, col i//width."""
    assert len(v) % width == 0
    return np.ascontiguousarray(v.reshape(-1, width).T)


def host_prep(inputs):
    src = np.asarray(inputs["src"]).astype(np.int64)
    dst = np.asarray(inputs["dst"]).astype(np.int64)
    glove = np.asarray(inputs["all_glove"], dtype=np.float32)
    N = glove.shape[0]
    E = src.shape[0]
    W1 = np.asarray(inputs["W1"], dtype=np.float32)
    FD = np.asarray(inputs["W_word"]).shape[0]
    IN = W1.shape[0]
    GLOVE = glove.shape[1]

    # norms (DGL norm='both', zero where degree zero)
    deg_out = np.bincount(src, minlength=N).astype(np.float32)
    deg_in = np.bincount(dst, minlength=N).astype(np.float32)
    ns = np.where(deg_out > 0, np.maximum(deg_out, 1.0) ** -0.5, 0.0).astype(np.float32)
    nd = np.where(deg_in > 0, np.maximum(deg_in, 1.0) ** -0.5, 0.0).astype(np.float32)
    rprime = np.bincount(dst, weights=ns[src].astype(np.float64), minlength=N).astype(np.float32)

    DCORE = N // P
    NB = math.ceil(DCORE / 128)
    # per-core slot lists: dedup by (block, src); each slot gathers one source
    # row and scatters to up to two dst columns (dloc_a, dloc_b). Slots with a
    # second dst are packed FIRST within each block so their extra matmuls are
    # confined to each block's first tiles (flagged below).
    cores = []
    maxtiles = 1
    for c in range(P):
        sel = (dst >= c * DCORE) & (dst < (c + 1) * DCORE)
        es, ed = src[sel], dst[sel] - c * DCORE
        blk = ed // 128
        key = blk * N + es
        order = np.argsort(key, kind="stable")
        es, ed, blk, key = es[order], ed[order], blk[order], key[order]
        run_start = np.r_[True, key[1:] != key[:-1]]
        run_id = np.cumsum(run_start) - 1
        starts = np.flatnonzero(run_start)
        pos_in_run = np.arange(len(es)) - starts[run_id]
        run_counts = np.bincount(run_id)
        cum = np.zeros(len(starts) + 1, np.int64)
        np.cumsum((run_counts + 1) // 2, out=cum[1:])
        slot = cum[run_id] + pos_in_run // 2
        nslots = int(cum[-1])
        s_src = np.zeros(nslots, np.int64)
        s_blk = np.zeros(nslots, np.int64)
        s_a = np.full(nslots, -1.0, np.float32)
        s_b = np.full(nslots, -1.0, np.float32)
        s_src[slot] = es
        s_blk[slot] = blk
        isa = pos_in_run % 2 == 0
        s_a[slot[isa]] = (ed[isa] - blk[isa] * 128).astype(np.float32)
        s_b[slot[~isa]] = (ed[~isa] - blk[~isa] * 128).astype(np.float32)
        # pack b-carrying slots first within each block (stable)
        reorder = np.argsort(s_blk * 2 + (s_b < 0), kind="stable")
        s_src, s_blk, s_a, s_b = s_src[reorder], s_blk[reorder], s_a[reorder], s_b[reorder]
        cnts = np.bincount(s_blk, minlength=NB)
        maxtiles = max(maxtiles, int(math.ceil(cnts.max() / 128)))
        cores.append((s_src, s_a, s_b, cnts))

    meta = Meta(N, E, FD, IN, GLOVE, maxtiles)
    m = meta

    per_core = []
    L = m.NTILE * 128
    anyb = np.zeros(L, bool)
    for c in range(P):
        s_src, s_a, s_b, cnts = cores[c]
        idx1 = np.zeros(L, np.int64)
        dloca = np.full(L, -1.0, np.float32)
        dlocb = np.full(L, -1.0, np.float32)
        off = 0
        for b in range(m.NB):
            nb = int(cnts[b])
            sl = slice(b * m.T_blk * 128, b * m.T_blk * 128 + nb)
            idx1[sl] = s_src[off:off + nb]
            dloca[sl] = s_a[off:off + nb]
            dlocb[sl] = s_b[off:off + nb]
            off += nb
        assert off == len(s_src)
        anyb |= dlocb >= 0
        idx3 = m.DPAD * (idx1 // m.DCORE) + idx1 % m.DCORE
        L3 = m.NTILE3 * 128
        if L3 > L:
            idx3 = np.concatenate([idx3, np.zeros(L3 - L, np.int64)])
        per_core.append(dict(
            idx1=_pack_idx(idx1.astype(np.int16)),
            idx3=_pack_idx(idx3[:L3].astype(np.int16)),
            dstloca=_pack_tile_major(dloca, -1.0),
            dstlocb=_pack_tile_major(dlocb, -1.0),
        ))

    # union-over-cores per-tile flag: does tile t carry any second-dst slot?
    tile_has_b = [bool(anyb[t * 128:(t + 1) * 128].any()) for t in range(m.NTILE)]
    # last tile of each block must not be flagged (stop flag sits on its a-MM)
    for b in range(m.NB):
        assert not tile_has_b[b * m.T_blk + m.T_blk - 1] or m.T_blk == 1
    m.tile_has_b = tile_has_b

    prep = dict(meta=m, ns=ns, nd=nd, rprime=rprime, per_core=per_core)
    return prep


def host_arrays(inputs, prep):
    """Build all device input arrays (shared + per-core)."""
    m = prep["meta"]
    ns, nd, rprime = prep["ns"], prep["nd"], prep["rprime"]
    f32 = np.float32

    glove = np.asarray(inputs["all_glove"], dtype=f32)
    W_word = np.asarray(inputs["W_word"], dtype=f32)   # [FD, GLOVE]
    b_word = np.asarray(inputs["b_word"], dtype=f32)
    W_img = np.asarray(inputs["W_img"], dtype=f32)     # [FD, N]
    b_img = np.asarray(inputs["b_img"], dtype=f32)
    ce = np.asarray(inputs["class_embed"], dtype=f32)
    W1 = np.asarray(inputs["W1"], dtype=f32)
    b1 = np.asarray(inputs["b1"], dtype=f32)
    W2 = np.asarray(inputs["W2"], dtype=f32)
    b2 = np.asarray(inputs["b2"], dtype=f32)
    W3 = np.asarray(inputs["W3"], dtype=f32)           # [IN, 1]
    b3 = np.asarray(inputs["b3"], dtype=f32)
    W_fin = np.asarray(inputs["W_fin"], dtype=f32)     # [FD, N]

    # gloveT padded [GLP, NPAD] bf16
    gloveT = np.zeros((m.GLP, m.NPAD), f32)
    gloveT[:m.GLOVE, :m.N] = glove.T
    # W_wordT packed [128, NGC*F1]
    wwT = np.zeros((m.GLP, m.F1), f32)
    wwT[:m.GLOVE, :m.FD] = W_word.T
    wwordT = wwT.reshape(m.NGC, 128, m.F1).transpose(1, 0, 2).reshape(128, -1)
    bword_row = np.zeros((1, m.F1), f32)
    bword_row[0, :m.FD] = b_word + 0.0
    # fold b_img into... ce_vec = W_img @ ce + b_img; do bias via extra contraction row:
    # instead append b_img as one extra "node" with ce value 1.
    wimgT = np.zeros((m.NPAD, m.F1), f32)
    wimgT[:m.N, :m.FD] = W_img.T
    ce_pm = np.zeros((128, m.NT_N), f32)
    cev = np.zeros(m.NPAD, f32)
    cev[:m.N] = ce
    # bias row trick: use padded node N as constant-1 with weight b_img
    wimgT[m.N, :m.FD] = b_img
    cev[m.N] = 1.0
    ce_pm[:, :] = cev.reshape(m.NT_N, 128).T

    # W1 split
    W1_top = np.zeros((m.F1, m.F2), f32)
    W1_top[:m.FD, :m.IN] = W1[:m.FD, :]
    W1_bot = np.zeros((m.F1, m.F2), f32)
    W1_bot[:m.FD, :m.IN] = W1[m.FD:, :]
    w1t = W1_top.reshape(m.NC1, 128, m.F2).transpose(1, 0, 2).reshape(128, -1)
    w1b = (W1_bot.reshape(m.NC1, 128, m.NC2, 128)
           .transpose(1, 0, 2, 3).reshape(128, -1))
    W2p = np.zeros((m.F2, m.F2), f32)
    W2p[:m.IN, :m.IN] = W2
    w2 = (W2p.reshape(m.NC2, 128, m.NC2, 128)
          .transpose(1, 0, 2, 3).reshape(128, -1))
    b1_pm = np.zeros((128, m.NC2), f32)
    b1_pm[:, :] = np.pad(b1, (0, m.F2 - m.IN)).reshape(m.NC2, 128).T
    b2_pm = np.zeros((128, m.NC2), f32)
    b2_pm[:, :] = np.pad(b2, (0, m.F2 - m.IN)).reshape(m.NC2, 128).T
    w3_pm = np.pad(W3[:, 0], (0, m.F2 - m.IN)).reshape(m.NC2, 128).T.copy()
    b3_bc = np.full((128, 1), b3[0], f32)

    ns_pm = np.zeros((128, m.NT_N), f32)
    ns_pm[:, :] = np.pad(ns, (0, m.NPAD - m.N)).reshape(m.NT_N, 128).T

    iota = np.tile(np.arange(128, dtype=f32), (128, 1))
    id_f32 = np.eye(128, dtype=f32)
    id_bf = np.eye(128, dtype=f32).astype(BF16)
    ones_row = np.ones((1, 128), f32).astype(BF16)

    shared1 = dict(
        gloveT=gloveT.astype(BF16), wwordT=wwordT.astype(BF16),
        bword_row=bword_row.astype(BF16), ones_row=ones_row,
        wimgT=wimgT, ce_pm=ce_pm,
        w1t=w1t, w1b=w1b, b1_pm=b1_pm, ns_pm=ns_pm,
        iota=iota, id_f32=id_f32, id_bf=id_bf,
    )
    shared2 = dict(
        w2=w2, b2_pm=b2_pm, w3_pm=w3_pm, b3_bc=b3_bc,
        iota=iota, id_f32=id_f32,
    )

    W_finT = W_fin.T  # [N, FD]
    per1, per2 = [], []
    for c in range(P):
        pc = prep["per_core"][c]
        ndc = np.zeros(m.DPAD, f32)
        ndc[:m.DCORE] = nd[c * m.DCORE:(c + 1) * m.DCORE]
        nsc = np.zeros(m.DPAD, f32)
        nsc[:m.DCORE] = ns[c * m.DCORE:(c + 1) * m.DCORE]
        rpc = np.zeros((1, m.DPAD), f32)
        rpc[0, :m.DCORE] = rprime[c * m.DCORE:(c + 1) * m.DCORE]
        nd_bc = np.tile(ndc, (128, 1))
        ns_bc = np.tile(nsc, (128, 1))
        nd_dstpm = ndc.reshape(m.NB, 128).T.copy()
        wfc = np.zeros((m.DPAD, m.F1), f32)
        wfc[:m.DCORE, :m.FD] = W_finT[c * m.DCORE:(c + 1) * m.DCORE, :]
        wfin = (wfc.reshape(m.NB, 128, m.NC1, 128)
                .transpose(1, 0, 2, 3).reshape(128, -1))
        per1.append(dict(
            idx1=pc["idx1"], dstloca=pc["dstloca"], dstlocb=pc["dstlocb"],
            nd_bc=nd_bc, ns_bc=ns_bc, rp_row=rpc,
        ))
        per2.append(dict(
            idx1=pc["idx1"], idx3=pc["idx3"], dstloca=pc["dstloca"],
            dstlocb=pc["dstlocb"],
            nd_bc=nd_bc, ns_bc=ns_bc, nd_dstpm=nd_dstpm, wfin=wfin,
        ))
    return shared1, per1, shared2, per2


# ------------------------------------------------------------- device progs

def _const_load(nc, pool, name_ap, shape, dtype):
    t = pool.tile(shape, dtype, tag=f"c_{name_ap.name}")
    nc.sync.dma_start(t[:], name_ap[:])
    return t


def _spmm(nc, tc, m, src_ap, idx_sb, dloca_sb, dlocb_sb, iota_sb, elem, gdt, oh_dt,
          agg_pool, agg_shape, gbufs, chunk_tiles, nchunks, block_end, tag):
    """Gather + one-hot scatter matmul over all blocks.

    block_end(b, psum_agg) is called after the last accumulate of block b.
    """
    import os
    spmm_lvl = int(os.environ.get("K_SPMM", "3"))
    Gc = chunk_tiles
    with tc.tile_pool(name=f"g_{tag}", bufs=gbufs) as gpool, \
         tc.tile_pool(name=f"oh_{tag}", bufs=OBUFS) as opool:
        psum_agg = None
        for ch in range(nchunks):
            g = gpool.tile([128, Gc, elem], gdt)
            nc.gpsimd.dma_gather(
                g[:], src_ap,
                idx_sb[:, ch * Gc * 8:(ch + 1) * Gc * 8],
                Gc * 128, Gc * 128, elem)
            if spmm_lvl < 2:
                continue
            oh = opool.tile([128, Gc, 128], oh_dt)
            nc.vector.tensor_tensor(
                oh[:],
                dloca_sb[:, ch * Gc:(ch + 1) * Gc].unsqueeze(-1).broadcast_to([128, Gc, 128]),
                iota_sb[:].unsqueeze(1).broadcast_to([128, Gc, 128]),
                op=ALU.is_equal)
            flags = [m.tile_has_b[min(ch * Gc + j, m.NTILE - 1)] for j in range(Gc)]
            ohb = None
            if any(flags):
                ohb = opool.tile([128, Gc, 128], oh_dt, tag=f"ohb_{tag}")
                nc.vector.tensor_tensor(
                    ohb[:],
                    dlocb_sb[:, ch * Gc:(ch + 1) * Gc].unsqueeze(-1).broadcast_to([128, Gc, 128]),
                    iota_sb[:].unsqueeze(1).broadcast_to([128, Gc, 128]),
                    op=ALU.is_equal)
            if spmm_lvl < 3:
                continue
            for j in range(Gc):
                t = ch * Gc + j
                if t >= m.ntiles:
                    break
                b, k = t // m.T_blk, t % m.T_blk
                if k == 0:
                    psum_agg = agg_pool.tile(agg_shape, dt.float32, space="PSUM")
                nc.tensor.matmul(psum_agg[:], lhsT=oh[:, j, :], rhs=g[:, j, :],
                                 start=(k == 0), stop=(k == m.T_blk - 1))
                if flags[j]:
                    nc.tensor.matmul(psum_agg[:], lhsT=ohb[:, j, :], rhs=g[:, j, :],
                                     start=False, stop=False)
                if k == m.T_blk - 1:
                    block_end(b, psum_agg)


def _fin(nc):
    return None


def build_launch1(m: Meta):
    nc = bacc.Bacc("TRN2", debug=False, target_bir_lowering=False, num_devices=P)
    f32, bf16, i16 = dt.float32, dt.bfloat16, dt.int16

    gloveT = nc.dram_tensor("gloveT", [m.GLP, m.NPAD], bf16, kind="ExternalInput")
    wwordT = nc.dram_tensor("wwordT", [128, m.NGC * m.F1], bf16, kind="ExternalInput")
    bword_row = nc.dram_tensor("bword_row", [1, m.F1], bf16, kind="ExternalInput")
    ones_row = nc.dram_tensor("ones_row", [1, 128], bf16, kind="ExternalInput")
    wimgT = nc.dram_tensor("wimgT", [m.NPAD, m.F1], f32, kind="ExternalInput")
    ce_pm = nc.dram_tensor("ce_pm", [128, m.NT_N], f32, kind="ExternalInput")
    w1t = nc.dram_tensor("w1t", [128, m.NC1 * m.F2], f32, kind="ExternalInput")
    w1b = nc.dram_tensor("w1b", [128, m.NC1 * m.NC2 * 128], f32, kind="ExternalInput")
    b1_pm = nc.dram_tensor("b1_pm", [128, m.NC2], f32, kind="ExternalInput")
    ns_pm = nc.dram_tensor("ns_pm", [128, m.NT_N], f32, kind="ExternalInput")
    nd_bc = nc.dram_tensor("nd_bc", [128, m.DPAD], f32, kind="ExternalInput")
    ns_bc = nc.dram_tensor("ns_bc", [128, m.DPAD], f32, kind="ExternalInput")
    rp_row = nc.dram_tensor("rp_row", [1, m.DPAD], f32, kind="ExternalInput")
    idx1 = nc.dram_tensor("idx1", [128, m.NTILE * 8], i16, kind="ExternalInput")
    dstloca = nc.dram_tensor("dstloca", [128, m.NTILE], f32, kind="ExternalInput")
    dstlocb = nc.dram_tensor("dstlocb", [128, m.NTILE], f32, kind="ExternalInput")
    iota = nc.dram_tensor("iota", [128, 128], f32, kind="ExternalInput")
    id_f32 = nc.dram_tensor("id_f32", [128, 128], f32, kind="ExternalInput")
    id_bf = nc.dram_tensor("id_bf", [128, 128], bf16, kind="ExternalInput")

    h1s_mine = nc.dram_tensor("h1s_mine", [m.DCORE, m.F2], bf16, kind="ExternalOutput")
    import os as _os
    if _os.environ.get("K_EXTSRC"):
        we_s = nc.dram_tensor("we_s", [m.NPAD, m.F1], bf16, kind="ExternalInput")
    else:
        we_s = nc.dram_tensor("we_s", [m.NPAD, m.F1], bf16)

    with tile.TileContext(nc) as tc:
        with tc.tile_pool(name="const", bufs=1) as cp:
            wwordT_sb = _const_load(nc, cp, wwordT, [128, m.NGC * m.F1], bf16)
            bword_sb = _const_load(nc, cp, bword_row, [1, m.F1], bf16)
            ones_sb = _const_load(nc, cp, ones_row, [1, 128], bf16)
            ce_sb = _const_load(nc, cp, ce_pm, [128, m.NT_N], f32)
            w1t_sb = _const_load(nc, cp, w1t, [128, m.NC1 * m.F2], f32)
            w1b_sb = _const_load(nc, cp, w1b, [128, m.NC1 * m.NC2 * 128], f32)
            b1_sb = _const_load(nc, cp, b1_pm, [128, m.NC2], f32)
            nspm_sb = _const_load(nc, cp, ns_pm, [128, m.NT_N], f32)
            ndbc_sb = _const_load(nc, cp, nd_bc, [128, m.DPAD], f32)
            nsbc_sb = _const_load(nc, cp, ns_bc, [128, m.DPAD], f32)
            rp_sb = _const_load(nc, cp, rp_row, [1, m.DPAD], f32)
            idx1_sb = _const_load(nc, cp, idx1, [128, m.NTILE * 8], i16)
            dloca_sb = _const_load(nc, cp, dstloca, [128, m.NTILE], f32)
            dlocb_sb = _const_load(nc, cp, dstlocb, [128, m.NTILE], f32)
            iota_sb = _const_load(nc, cp, iota, [128, 128], f32)
            idf_sb = _const_load(nc, cp, id_f32, [128, 128], f32)
            idb_sb = _const_load(nc, cp, id_bf, [128, 128], bf16)
            u_sb = cp.tile([1, m.F2], f32)
            cecol_sb = cp.tile([128, m.NC1], f32)

            # ---------------- phase A1: we_s table
            with tc.tile_pool(name="glove", bufs=3) as glp, \
                 tc.tile_pool(name="wesb", bufs=3) as wep, \
                 tc.tile_pool(name="psA", bufs=PSA, space="PSUM") as psA:
                for jg in range(m.NT_G):
                    gl = glp.tile([128, m.NGC, 512], bf16)
                    nc.sync.dma_start(
                        gl[:], gloveT[:, jg * 512:(jg + 1) * 512]
                        .rearrange("(c p) u -> p c u", p=128))
                    web = wep.tile([128, 4, m.F1], bf16)
                    for tsub in range(4):
                        nt = jg * 4 + tsub
                        pw = psA.tile([128, m.F1], f32, space="PSUM")
                        for ci in range(m.NGC):
                            nc.tensor.matmul(
                                pw[:],
                                lhsT=gl[:, ci, tsub * 128:(tsub + 1) * 128],
                                rhs=wwordT_sb[:, ci * m.F1:(ci + 1) * m.F1],
                                start=(ci == 0), stop=False)
                        nc.tensor.matmul(pw[:], lhsT=ones_sb[:1, :],
                                         rhs=bword_sb[:1, :], start=False, stop=True)
                        nc.scalar.mul(web[:, tsub, :], pw[:], nspm_sb[:, nt:nt + 1])
                    if not _os.environ.get("K_EXTSRC"):
                        nc.sync.dma_start(
                            we_s[jg * 512:(jg + 1) * 512, :]
                            .rearrange("(c p) f -> p c f", p=128),
                            web[:])

            if not MERGEA:
                tc.no_sync_barrier()
            import os
            stage = int(os.environ.get("K_STAGE", "9"))

            if stage >= 2:
                # ---------------- phase A2: ce_vec + u
                with tc.tile_pool(name="wimg", bufs=3) as wip, \
                     tc.tile_pool(name="psCE", bufs=1, space="PSUM") as psCE:
                    psum_ce = psCE.tile([128, m.NC1], f32, space="PSUM")
                    for jg in range(m.NT_G):
                        wi = wip.tile([128, 4, m.F1], f32)
                        nc.sync.dma_start(
                            wi[:], wimgT[jg * 512:(jg + 1) * 512, :]
                            .rearrange("(c p) f -> p c f", p=128))
                        for tsub in range(4):
                            nt = jg * 4 + tsub
                            for cc in range(m.NC1):
                                nc.tensor.matmul(
                                    psum_ce[:, cc:cc + 1],
                                    lhsT=wi[:, tsub, cc * 128:(cc + 1) * 128],
                                    rhs=ce_sb[:, nt:nt + 1],
                                    start=(nt == 0 and cc == 0),
                                    stop=(nt == m.NT_N - 1 and cc == m.NC1 - 1))
                    nc.vector.tensor_copy(cecol_sb[:], psum_ce[:])
                    pu = psCE.tile([1, m.F2], f32, space="PSUM")
                    for ci in range(m.NC1):
                        nc.tensor.matmul(pu[:], lhsT=cecol_sb[:, ci:ci + 1],
                                         rhs=w1t_sb[:, ci * m.F2:(ci + 1) * m.F2],
                                         start=(ci == 0), stop=(ci == m.NC1 - 1))
                    nc.vector.tensor_copy(u_sb[:], pu[:])

            tc.no_sync_barrier()

            if stage >= 3:
                # ---------------- phase B: layer-1 SpMM + W1 + epilogue
                with tc.tile_pool(name="psAgg", bufs=2, space="PSUM") as pAgg, \
                     tc.tile_pool(name="psT", bufs=PT1, space="PSUM") as pT, \
                     tc.tile_pool(name="psH", bufs=PH1, space="PSUM") as pH, \
                     tc.tile_pool(name="psHT", bufs=PHT1, space="PSUM") as pHT, \
                     tc.tile_pool(name="blk", bufs=BLKB) as bp:

                    def block_end_trivial(b, psum_agg):
                        agg_sb = bp.tile([128, m.F1], f32, tag="agg_sb")
                        nc.vector.tensor_copy(agg_sb[:], psum_agg[:])
                        hv = bp.tile([128, m.F1], bf16, tag="hv")
                        nc.vector.tensor_copy(hv[:], agg_sb[:])
                        dv = min(128, m.DCORE - b * 128)
                        nc.sync.dma_start(h1s_mine[b * 128:b * 128 + dv, :m.F1], hv[:dv, :])

                    def block_end(b, psum_agg):
                        agg_sb = bp.tile([128, m.F1], f32, tag="agg_sb")
                        nc.vector.tensor_copy(agg_sb[:], psum_agg[:])
                        paggT = pT.tile([128, m.F1], f32, space="PSUM")
                        for ci in range(m.NC1):
                            nc.tensor.transpose(paggT[:, ci * 128:(ci + 1) * 128],
                                                agg_sb[:, ci * 128:(ci + 1) * 128], idf_sb[:])
                        aggT_sb = bp.tile([128, m.F1], f32, tag="aggT_sb")
                        nc.vector.tensor_copy(aggT_sb[:], paggT[:])
                        ph = pH.tile([128, m.F2], f32, space="PSUM")
                        for co in range(m.NC2):
                            for ci in range(m.NC1):
                                nc.tensor.matmul(
                                    ph[:, co * 128:(co + 1) * 128],
                                    lhsT=w1b_sb[:, (ci * m.NC2 + co) * 128:(ci * m.NC2 + co + 1) * 128],
                                    rhs=aggT_sb[:, ci * 128:(ci + 1) * 128],
                                    start=(co == 0 and ci == 0), stop=False)
                            nc.tensor.matmul(
                                ph[:, co * 128:(co + 1) * 128],
                                lhsT=u_sb[:1, co * 128:(co + 1) * 128],
                                rhs=rp_sb[:1, b * 128:(b + 1) * 128],
                                start=False, stop=(co == m.NC2 - 1))
                        # epilogue: relu(nd*ph + b1) * ns, cast bf16 (h1sT)
                        h1sT = bp.tile([128, m.F2], bf16, tag="h1sT")
                        for co in range(m.NC2):
                            t1 = bp.tile([128, 128], f32, tag="t1")
                            nc.vector.tensor_tensor(
                                t1[:], ph[:, co * 128:(co + 1) * 128],
                                ndbc_sb[:, b * 128:(b + 1) * 128], op=ALU.mult)
                            t2 = bp.tile([128, 128], f32, tag="t2")
                            nc.scalar.activation(t2[:], t1[:], AF.Relu,
                                                 bias=b1_sb[:, co:co + 1])
                            nc.vector.tensor_tensor(
                                h1sT[:, co * 128:(co + 1) * 128], t2[:],
                                nsbc_sb[:, b * 128:(b + 1) * 128], op=ALU.mult)
                        pht = pHT.tile([128, m.F2], bf16, space="PSUM")
                        for co in range(m.NC2):
                            nc.tensor.transpose(pht[:, co * 128:(co + 1) * 128],
                                                h1sT[:, co * 128:(co + 1) * 128], idb_sb[:])
                        hrows = bp.tile([128, m.F2], bf16, tag="hrows")
                        nc.vector.tensor_copy(hrows[:], pht[:])
                        dv = min(128, m.DCORE - b * 128)
                        nc.sync.dma_start(h1s_mine[b * 128:b * 128 + dv, :], hrows[:dv, :])

                    _spmm(nc, tc, m, we_s[:], idx1_sb, dloca_sb, dlocb_sb, iota_sb,
                          m.F1, bf16, bf16, pAgg, [128, m.F1], GBUFS, G, m.nch,
                          block_end if stage >= 4 else block_end_trivial, "l1")
    nc.compile()
    return nc


def build_launch2(m: Meta):
    nc = bacc.Bacc("TRN2", debug=False, target_bir_lowering=False, num_devices=P)
    f32, bf16, i16 = dt.float32, dt.bfloat16, dt.int16

    h1s = nc.dram_tensor("h1s", [m.N, m.F2], bf16, kind="ExternalInput")
    w2 = nc.dram_tensor("w2", [128, m.NC2 * m.NC2 * 128], f32, kind="ExternalInput")
    b2_pm = nc.dram_tensor("b2_pm", [128, m.NC2], f32, kind="ExternalInput")
    w3_pm = nc.dram_tensor("w3_pm", [128, m.NC2], f32, kind="ExternalInput")
    b3_bc = nc.dram_tensor("b3_bc", [128, 1], f32, kind="ExternalInput")
    nd_bc = nc.dram_tensor("nd_bc", [128, m.DPAD], f32, kind="ExternalInput")
    ns_bc = nc.dram_tensor("ns_bc", [128, m.DPAD], f32, kind="ExternalInput")
    nd_dstpm = nc.dram_tensor("nd_dstpm", [128, m.NB], f32, kind="ExternalInput")
    wfin = nc.dram_tensor("wfin", [128, m.NB * m.F1], f32, kind="ExternalInput")
    idx1 = nc.dram_tensor("idx1", [128, m.NTILE * 8], i16, kind="ExternalInput")
    idx3 = nc.dram_tensor("idx3", [128, m.NTILE3 * 8], i16, kind="ExternalInput")
    dstloca = nc.dram_tensor("dstloca", [128, m.NTILE], f32, kind="ExternalInput")
    dstlocb = nc.dram_tensor("dstlocb", [128, m.NTILE], f32, kind="ExternalInput")
    iota = nc.dram_tensor("iota", [128, 128], f32, kind="ExternalInput")
    id_f32 = nc.dram_tensor("id_f32", [128, 128], f32, kind="ExternalInput")

    fin_out = nc.dram_tensor("fin_out", [128, m.NC1], f32, kind="ExternalOutput")
    z_mine = nc.dram_tensor("z_mine", [m.DPAD], f32)
    z_all = nc.dram_tensor("z_all", [P * m.DPAD], f32, addr_space="Shared")
    z_rep = nc.dram_tensor("z_rep", [P * m.DPAD * 64], f32)

    with tile.TileContext(nc) as tc:
        with tc.tile_pool(name="const", bufs=1) as cp:
            w2_sb = _const_load(nc, cp, w2, [128, m.NC2 * m.NC2 * 128], f32)
            b2_sb = _const_load(nc, cp, b2_pm, [128, m.NC2], f32)
            w3_sb = _const_load(nc, cp, w3_pm, [128, m.NC2], f32)
            b3_sb = _const_load(nc, cp, b3_bc, [128, 1], f32)
            ndbc_sb = _const_load(nc, cp, nd_bc, [128, m.DPAD], f32)
            nsbc_sb = _const_load(nc, cp, ns_bc, [128, m.DPAD], f32)
            nddst_sb = _const_load(nc, cp, nd_dstpm, [128, m.NB], f32)
            wfin_sb = _const_load(nc, cp, wfin, [128, m.NB * m.F1], f32)
            idx1_sb = _const_load(nc, cp, idx1, [128, m.NTILE * 8], i16)
            idx3_sb = _const_load(nc, cp, idx3, [128, m.NTILE3 * 8], i16)
            dloca_sb = _const_load(nc, cp, dstloca, [128, m.NTILE], f32)
            dlocb_sb = _const_load(nc, cp, dstlocb, [128, m.NTILE], f32)
            iota_sb = _const_load(nc, cp, iota, [128, 128], f32)
            idf_sb = _const_load(nc, cp, id_f32, [128, 128], f32)
            z_sb = cp.tile([1, m.DPAD], f32)
            o_sb = cp.tile([128, m.NB], f32)

            # ---------------- layer-2 SpMM + W2 + z
            with tc.tile_pool(name="psAgg", bufs=PAGG2, space="PSUM") as pAgg, \
                 tc.tile_pool(name="psT", bufs=PT2, space="PSUM") as pT, \
                 tc.tile_pool(name="psH", bufs=PH2, space="PSUM") as pH, \
                 tc.tile_pool(name="psZ", bufs=PZ2, space="PSUM") as pZ, \
                 tc.tile_pool(name="blk", bufs=BLKB) as bp:

                def block_end2(b, psum_agg):
                    agg_sb = bp.tile([128, m.F2], f32, tag="agg_sb")
                    nc.vector.tensor_copy(agg_sb[:], psum_agg[:])
                    paggT = pT.tile([128, m.F2], f32, space="PSUM")
                    for ci in range(m.NC2):
                        nc.tensor.transpose(paggT[:, ci * 128:(ci + 1) * 128],
                                            agg_sb[:, ci * 128:(ci + 1) * 128], idf_sb[:])
                    aggT_sb = bp.tile([128, m.F2], f32, tag="aggT_sb")
                    nc.vector.tensor_copy(aggT_sb[:], paggT[:])
                    ph = pH.tile([128, m.F2], f32, space="PSUM")
                    for co in range(m.NC2):
                        for ci in range(m.NC2):
                            nc.tensor.matmul(
                                ph[:, co * 128:(co + 1) * 128],
                                lhsT=w2_sb[:, (ci * m.NC2 + co) * 128:(ci * m.NC2 + co + 1) * 128],
                                rhs=aggT_sb[:, ci * 128:(ci + 1) * 128],
                                start=(co == 0 and ci == 0),
                                stop=(co == m.NC2 - 1 and ci == m.NC2 - 1))
                    h2sT = bp.tile([128, m.F2], f32, tag="h2sT")
                    for co in range(m.NC2):
                        t1 = bp.tile([128, 128], f32, tag="t1")
                        nc.vector.tensor_tensor(
                            t1[:], ph[:, co * 128:(co + 1) * 128],
                            ndbc_sb[:, b * 128:(b + 1) * 128], op=ALU.mult)
                        t2 = bp.tile([128, 128], f32, tag="t2")
                        nc.scalar.activation(t2[:], t1[:], AF.Relu,
                                             bias=b2_sb[:, co:co + 1])
                        nc.vector.tensor_tensor(
                            h2sT[:, co * 128:(co + 1) * 128], t2[:],
                            nsbc_sb[:, b * 128:(b + 1) * 128], op=ALU.mult)
                    pz = pZ.tile([1, 128], f32, space="PSUM")
                    for co in range(m.NC2):
                        nc.tensor.matmul(pz[:], lhsT=w3_sb[:, co:co + 1],
                                         rhs=h2sT[:, co * 128:(co + 1) * 128],
                                         start=(co == 0), stop=(co == m.NC2 - 1))
                    nc.vector.tensor_copy(z_sb[:1, b * 128:(b + 1) * 128], pz[:])

                _spmm(nc, tc, m, h1s[:], idx1_sb, dloca_sb, dlocb_sb, iota_sb,
                      m.F2, bf16, bf16, pAgg, [128, m.F2], GBUFS, G, m.nch,
                      block_end2, "l2")

            if not NOBAR2:
                tc.no_sync_barrier()

            # ---------------- z exchange + z_rep build
            with tc.tile_pool(name="zx", bufs=1) as zp:
                nc.sync.dma_start(z_mine[:], z_sb[:])
                nc.gpsimd.collective_compute(
                    "AllGather", ALU.bypass,
                    replica_groups=[list(range(P))],
                    ins=[z_mine[:]], outs=[z_all[:]])
                zsb2 = zp.tile([128, m.ZT], f32)
                nc.sync.dma_start(
                    zsb2[:], z_all[:].rearrange("(p t) -> p t", p=128))
                zb = zp.tile([128, m.ZT, 64], f32)
                nc.vector.tensor_copy(
                    zb[:], zsb2[:].unsqueeze(-1).broadcast_to([128, m.ZT, 64]))
                nc.sync.dma_start(
                    z_rep[:].rearrange("(p q) -> p q", p=128),
                    zb[:].rearrange("p t j -> p (t j)"))

            if not NOBAR2:
                tc.no_sync_barrier()

            # ---------------- layer-3 SpMM + o + final partial
            with tc.tile_pool(name="psO", bufs=2, space="PSUM") as pO, \
                 tc.tile_pool(name="psF", bufs=1, space="PSUM") as pF, \
                 tc.tile_pool(name="fin", bufs=1) as fp:
                psum_fin = pF.tile([128, m.NC1], f32, space="PSUM")

                def block_end3(b, psum_o):
                    nc.scalar.activation(o_sb[:, b:b + 1], psum_o[:, 0:1], AF.Relu,
                                         bias=b3_sb[:, 0:1],
                                         scale=nddst_sb[:, b:b + 1])
                    for cc in range(m.NC1):
                        nc.tensor.matmul(
                            psum_fin[:, cc:cc + 1],
                            lhsT=wfin_sb[:, b * m.F1 + cc * 128:b * m.F1 + (cc + 1) * 128],
                            rhs=o_sb[:, b:b + 1],
                            start=(b == 0 and cc == 0),
                            stop=(b == m.NB - 1 and cc == m.NC1 - 1))

                _spmm(nc, tc, m, z_rep[:].rearrange("(r j) -> r j", j=64),
                      idx3_sb, dloca_sb, dlocb_sb, iota_sb,
                      64, f32, f32, pO, [128, 64], GBUFS, G3, m.nch3,
                      block_end3, "l3")
                fin_sb = fp.tile([128, m.NC1], f32)
                nc.vector.tensor_copy(fin_sb[:], psum_fin[:])
                nc.sync.dma_start(fin_out[:], fin_sb[:])
    nc.compile()
    return nc


# ---------------------------------------------------------------- kernel()

LAST_RESULTS = {}


def kernel(**inputs):
    prep = host_prep(inputs)
    m = prep["meta"]
    shared1, per1, shared2, per2 = host_arrays(inputs, prep)

    import os, time
    trace = bool(os.environ.get("BASS_TRACE"))
    t0 = time.time()
    nc1 = build_launch1(m)
    print(f"[kernel] launch1 built in {time.time()-t0:.1f}s", flush=True)
    if os.environ.get("K_EXTSRC"):
        shared1 = dict(shared1, we_s=np.zeros((m.NPAD, m.F1), BF16))
    in_maps1 = [dict(shared1, **per1[c]) for c in range(P)]
    t0 = time.time()
    r1 = run_bass_kernel_spmd(nc1, in_maps1, core_ids=list(range(P)), trace=trace)
    print(f"[kernel] launch1 ran in {time.time()-t0:.1f}s", flush=True)
    LAST_RESULTS["launch1"] = r1

    h1s_full = np.concatenate([r1.results[c]["h1s_mine"] for c in range(P)], axis=0)
    assert h1s_full.shape == (m.N, m.F2)
    if os.environ.get("K_STOP1"):
        print("[kernel] K_STOP1 set - stopping after launch1", flush=True)
        return np.zeros((1, m.FD), np.float32)

    t0 = time.time()
    nc2 = build_launch2(m)
    print(f"[kernel] launch2 built in {time.time()-t0:.1f}s", flush=True)
    shared2 = dict(shared2, h1s=np.ascontiguousarray(h1s_full))
    in_maps2 = [dict(shared2, **per2[c]) for c in range(P)]
    t0 = time.time()
    r2 = run_bass_kernel_spmd(nc2, in_maps2, core_ids=list(range(P)), trace=trace)
    print(f"[kernel] launch2 ran in {time.time()-t0:.1f}s", flush=True)
    LAST_RESULTS["launch2"] = r2

    # static HW-time estimate (no NTFF profiling hook available under axon):
    # per-core DMA bytes at ~360GB/s vs PE spans at 2.4GHz, max per launch.
    ebytes1 = m.ntiles * 128 * m.F1 * 2 + 44e6 / P * 0 + 24e6 + 20e6 + 10e6
    ebytes2 = m.ntiles * 128 * m.F2 * 2 + m.ntiles * 128 * 256 + 26e6
    est_ns = int((ebytes1 + ebytes2) / 360e9 * 1e9)
    print(f"[kernel] rough HW-time estimate (DMA-bound model): {est_ns} ns", flush=True)
    LAST_RESULTS["est_ns"] = est_ns

    b_fin = np.asarray(inputs["b_fin"], dtype=np.float32)
    total = np.zeros(m.F1, np.float32)
    for c in range(P):
        f = r2.results[c]["fin_out"].astype(np.float32)  # [128, NC1]
        total += f.T.reshape(-1)
    out = total[:m.FD] + b_fin
    return out.reshape(1, m.FD).astype(np.float32)

